# revision 9
# baseline (speedup 1.0000x reference)
"""Trainium2 Bass kernel for nn_MultiHeadAttention (dense transformer MHA).

Strategy (8-way tensor parallel over heads), v2:
  - Each of the 8 cores owns 2 heads (128 of the 1024 q/k/v features).
  - Host pre-transposes activations (query/key/value -> [D, T]), casts bf16;
    weights head-sliced per core (Wo full). RoPE is elementwise here
    (neg_half = [y1, -y2]) so it is one multiply by a host factor C^T.
  - Attention in transposed layout S^T[s, t]; unsafe softmax (exp on ACT,
    denominator via ones-column appended to V in the U matmul, normalize by
    partition-broadcast + multiply).
  - v2 scheduling: the TRN2 PE only reaches 2.4 GHz after ~3us of gapless
    execution and falls back to 1.2 GHz after any bubble, so the whole
    kernel is emitted as ONE continuous s-tile stream across all 4 t-chunks
    (64 tiles), with a lag-queue deferring each tile's U-matmuls a few tiles
    behind its logits (the ACT exp latency is hidden) and all other work
    (projections, chunk normalization, A2A ships, output projections)
    dropped into the stream as per-tile chores.  Inputs are DMA'd
    chunk-major (q/k) and s-major (v) on the two HWDGE rings so the first
    logits matmul can issue at ~9us instead of ~45us, and the PE is warmed
    up with throwaway matmuls until the first data lands.
  - Re-partition head-shard -> seq-shard with one AllToAll per 512-wide
    t-chunk; each core projects its own 64 rows per chunk through full Wo.
"""
import numpy as np
import ml_dtypes

import concourse.bass as bass
import concourse.mybir as mybir
import concourse.tile as tile
from concourse import bacc
from concourse.bass_utils import run_bass_kernel_spmd

# problem constants (hardcoded per contract)
T = 2048
D = 1024
H = 16
DH = 64
ROPE_BASE = 10000

N_CORES = 8
HPC = H // N_CORES          # heads per core = 2
FPC = HPC * DH              # features per core = 128
TC = 512                    # attention t-chunk
NTC = T // TC               # 4
NS = T // 128               # 16 s-tiles
ND = D // 128               # 8 d-tiles
VW = 2 * DH + 2             # 130: v_ext block width per s-tile
ROWS = TC // N_CORES        # 64 output rows per core per A2A chunk

bf16 = mybir.dt.bfloat16
f32 = mybir.dt.float32
EXP = mybir.ActivationFunctionType.Exp

_cache = {}


def _build(use_bias=True):
    nc = bacc.Bacc("TRN2", target_bir_lowering=False, debug=False,
                   num_devices=N_CORES)

    # ---- I/O -----------------------------------------------------------
    qT = nc.dram_tensor("qT", [D, T], bf16, kind="ExternalInput").ap()
    kT = nc.dram_tensor("kT", [D, T], bf16, kind="ExternalInput").ap()
    vT = nc.dram_tensor("vT", [D, T], bf16, kind="ExternalInput").ap()
    wq = nc.dram_tensor("wq", [D, FPC], bf16, kind="ExternalInput").ap()
    wk = nc.dram_tensor("wk", [D, FPC], bf16, kind="ExternalInput").ap()
    wv = nc.dram_tensor("wv", [D, FPC], bf16, kind="ExternalInput").ap()
    wo = nc.dram_tensor("wo", [D, D], bf16, kind="ExternalInput").ap()
    bq = nc.dram_tensor("bq", [1, FPC], bf16, kind="ExternalInput").ap()
    bk = nc.dram_tensor("bk", [1, FPC], bf16, kind="ExternalInput").ap()
    bv = nc.dram_tensor("bv", [1, FPC], bf16, kind="ExternalInput").ap()
    bo = nc.dram_tensor("bo", [1, D], bf16, kind="ExternalInput").ap()
    ropeC = nc.dram_tensor("ropeC", [FPC, T], f32, kind="ExternalInput").ap()
    outs = [nc.dram_tensor(f"out{q}", [ROWS, D], f32,
                           kind="ExternalOutput").ap() for q in range(NTC)]

    with tile.TileContext(nc) as tc:
        with (
            tc.tile_pool(name="win", bufs=1) as win,        # weights/consts
            tc.tile_pool(name="xin", bufs=1) as xin,        # input stream
            tc.tile_pool(name="qk", bufs=NTC) as qkpool,    # q^T / k^T
            tc.tile_pool(name="vx", bufs=NS) as vxpool,     # v_ext
            tc.tile_pool(name="ex", bufs=11) as expool,     # exp(S^T)
            tc.tile_pool(name="at", bufs=1) as atpool,      # attn^T halves
            tc.tile_pool(name="nrm", bufs=4) as nrmpool,    # u_sb / Rbc
            tc.tile_pool(name="opr", bufs=2) as oprpool,    # out-proj tiles
            tc.tile_pool(name="pp", bufs=2, space="PSUM") as pproj,
            tc.tile_pool(name="ps", bufs=2, space="PSUM") as pS,
            tc.tile_pool(name="pu", bufs=2, space="PSUM") as pU,
            tc.tile_pool(name="dram", bufs=1, space="DRAM") as dram,
        ):
            # ---- constants / weights / inputs, in consumption order ----
            wq_sb = win.tile([128, ND * FPC], bf16, tag="wq")
            wk_sb = win.tile([128, ND * FPC], bf16, tag="wk")
            wv_sb = win.tile([128, ND * FPC], bf16, tag="wv")
            bq_sb = win.tile([1, FPC], bf16, tag="bq")
            bk_sb = win.tile([1, FPC], bf16, tag="bk")
            bv_sb = win.tile([1, FPC], bf16, tag="bv")
            bo_sb = win.tile([1, D], bf16, tag="bo")
            ropes = [win.tile([FPC, TC], f32, tag="rope", bufs=NTC,
                              name=f"rope{i}") for i in range(NTC)]
            ones_sb = win.tile([1, T], bf16, tag="ones")
            nc.gpsimd.memset(ones_sb[:], 1.0)
            onesf_sb = win.tile([1, DH], f32, tag="onesf")
            nc.gpsimd.memset(onesf_sb[:], 1.0)
            # preload the EXP activation table so the first real exp in the
            # s-stream doesn't eat the ~1.3us table load.
            pre_sb = win.tile([1, 2], f32, tag="pre")
            nc.scalar.activation(pre_sb[:], onesf_sb[:, 0:2], EXP)
            qin = xin.tile([128, ND * T], bf16, tag="qin")
            kin = xin.tile([128, ND * T], bf16, tag="kin")
            vin = xin.tile([128, ND * T], bf16, tag="vin")

            # ---- input DMA, chunk-major so compute starts early --------
            # ring A = SP (sync), ring B = ACT (scalar); they drain
            # concurrently.  k gets a dedicated ring: every s-tile of chunk
            # c's logits needs k-chunk s//4, q/v/weights share ring A.
            def _wdma(eng, w_sb, w):
                eng.dma_start(
                    out=w_sb[:].rearrange("p (d m) -> p d m", d=ND),
                    in_=w.rearrange("(d p) m -> p d m", p=128))

            def _xchunk(eng, x_sb, x, c, nch=1):
                # one 512-wide column chunk (nch of them) of all 8 d-tiles,
                # as a SINGLE dma_start: keeping the per-ring outstanding-DMA
                # count low avoids issue-stalls on the initiating engine.
                cs = slice(TC * c, TC * (c + nch))
                eng.dma_start(
                    out=x_sb[:].rearrange("p (d m) -> p d m", d=ND)[:, :, cs],
                    in_=x.rearrange("(d p) m -> p d m", p=128)[:, :, cs])

            # ring A (SP): wq, bq, rope0, q0, wv, bv, v0, v1, q1, v2, v3,
            # q2, q3, bo.  ring B (ACT) prologue carries ONLY 4 DMAs
            # (wk, bk, k0, k1): the tile framework flow-controls each ring
            # to ~4 in-flight DMAs, so a longer ACT chain would block the
            # exp activations queued behind it on the ACT engine.  The
            # remaining ring-B loads (k2, k3, rope1-3, wo) are issued as
            # mid-stream chores between exps once the ring has drained.
            _wdma(nc.sync, wq_sb, wq)
            nc.sync.dma_start(out=bq_sb[:], in_=bq)
            nc.sync.dma_start(out=ropes[0][:], in_=ropeC[:, 0:TC])
            _xchunk(nc.sync, qin, qT, 0)
            _wdma(nc.sync, wv_sb, wv)
            nc.sync.dma_start(out=bv_sb[:], in_=bv)
            _xchunk(nc.sync, vin, vT, 0)
            _xchunk(nc.sync, vin, vT, 1)
            _xchunk(nc.sync, qin, qT, 1)
            _xchunk(nc.sync, vin, vT, 2)
            _xchunk(nc.sync, vin, vT, 3)
            _xchunk(nc.sync, qin, qT, 2)
            _xchunk(nc.sync, qin, qT, 3)
            nc.sync.dma_start(out=bo_sb[:], in_=bo)
            _wdma(nc.scalar, wk_sb, wk)
            nc.scalar.dma_start(out=bk_sb[:], in_=bk)
            _xchunk(nc.scalar, kin, kT, 0)
            _xchunk(nc.scalar, kin, kT, 1)
            wo_sb = win.tile([128, ND * D], bf16, tag="wo")

            def late_dma(what):
                if what == "wo":
                    nc.scalar.dma_start(
                        out=wo_sb[:].rearrange("p (d m) -> p d m", d=ND),
                        in_=wo.rearrange("(d p) m -> p d m", p=128))
                elif what.startswith("k"):
                    _xchunk(nc.scalar, kin, kT, int(what[1]))
                else:  # ropeN
                    i = int(what[4])
                    nc.scalar.dma_start(out=ropes[i][:],
                                        in_=ropeC[:, TC * i:TC * (i + 1)])

            # PE warmup: back-to-back matmuls until the first inputs land;
            # keeps the DVFS ramp running so projections start at speed.
            wup = pproj.tile([DH, 512], f32, tag="pp", name="wup")
            for _ in range(10):
                nc.tensor.matmul(wup[:], ones_sb[:, 0:DH], ones_sb[:, 0:512],
                                 start=True, stop=True)
            # consume the warmup result (it is exactly 1.0) so DCE keeps it
            nc.vector.tensor_copy(ones_sb[:, 0:512], wup[0:1, :])

            # ---- projections (per 512-wide chunk, chore-schedulable) ---
            qts = [qkpool.tile([128, TC], bf16, tag="qt", name=f"qt{i}")
                   for i in range(NTC)]
            kts = [qkpool.tile([128, TC], bf16, tag="kt", name=f"kt{i}")
                   for i in range(NTC)]

            def proj_chunk(which, c):
                x_sb, w_sb, b_sb, x_in = {
                    "q": (qts[c], wq_sb, bq_sb, qin),
                    "k": (kts[c], wk_sb, bk_sb, kin),
                }[which]
                ts = slice(TC * c, TC * (c + 1))
                ps = pproj.tile([128, TC], f32, tag="pp",
                                name=f"pj_{which}{c}")
                for d in range(ND):
                    nc.tensor.matmul(
                        ps[:], w_sb[:, FPC * d:FPC * (d + 1)],
                        x_in[:, T * d + TC * c:T * d + TC * (c + 1)],
                        start=(d == 0),
                        stop=(not use_bias and d == ND - 1))
                if use_bias:
                    nc.tensor.matmul(ps[:], b_sb[:], ones_sb[:, ts],
                                     start=False, stop=True)
                nc.vector.tensor_mul(x_sb[:], ps[:], ropes[c][:])

            # v_ext: 16 tiles [128, VW]; block: [v_h0 | ones | v_h1 | ones]
            vs = [vxpool.tile([128, VW], bf16, tag="vext", name=f"vext{s}")
                  for s in range(NS)]
            for s in range(NS):
                nc.gpsimd.memset(vs[s][:, DH::DH + 1], 1.0)  # ones columns

            def vproj(s):
                ps = pproj.tile([128, FPC], f32, tag="pp", name=f"vps{s}")
                for d in range(ND):
                    nc.tensor.matmul(
                        ps[:], vin[:, T * d + 128 * s:T * d + 128 * (s + 1)],
                        wv_sb[:, FPC * d:FPC * (d + 1)],
                        start=(d == 0),
                        stop=(not use_bias and d == ND - 1))
                if use_bias:
                    nc.tensor.matmul(ps[:], ones_sb[:, 0:128], bv_sb[:],
                                     start=False, stop=True)
                nc.vector.tensor_copy(
                    vs[s][:].rearrange("p (h w) -> p h w", h=2)[:, :, 0:DH],
                    ps.rearrange("p (h w) -> p h w", h=2))

            # ---- A2A bounce buffers ------------------------------------
            a2a_in = [dram.tile([8 * 128, ROWS], bf16, tag=f"a2ai{i}",
                                name=f"a2a_in{i}") for i in range(NTC)]
            a2a_out = [dram.tile([8 * 128, ROWS], bf16, tag=f"a2ao{i}",
                                 name=f"a2a_out{i}") for i in range(NTC)]

            # per-chunk U psum tiles, created lazily at first u-matmul
            ups = {}

            def u_mms(q, sl, ex):
                if q not in ups:
                    ups[q] = [pU.tile([DH + 1, TC], f32, tag="pu",
                                      name=f"up{q}_{h}") for h in range(HPC)]
                for h in range(HPC):
                    o = (DH + 1) * h
                    nc.tensor.matmul(
                        ups[q][h][:], vs[sl][:, o:o + DH + 1],
                        ex[:, TC * h:TC * (h + 1)],
                        start=(sl == 0), stop=(sl == NS - 1))

            # phase A: stage U and 1/colsum to SBUF, freeing PSUM slots
            nstate = {}

            def phase_a(q):
                u64, rr = [], []
                for h in range(HPC):
                    u_sb = nrmpool.tile([DH, TC], f32, tag="u64",
                                        name=f"u64_{q}_{h}")
                    nc.vector.tensor_copy(u_sb[:], ups[q][h][0:DH, :])
                    r_sb = nrmpool.tile([1, TC], f32, tag="rsb",
                                        name=f"rsb{q}_{h}")
                    nc.vector.tensor_copy(r_sb[:], ups[q][h][DH:DH + 1, :])
                    nc.vector.reciprocal_approx_fast(r_sb[:], r_sb[:])
                    u64.append(u_sb)
                    rr.append(r_sb)
                del ups[q]
                nstate[q] = (u64, rr, [None, None])

            # phase B: normalize per head (partition-broadcast of 1/sum via
            # a rank-1 matmul), then ship to the bounce + trigger the A2A.
            def pb_h(q, h):
                u64, rr, aTs = nstate[q]
                rbp = pproj.tile([DH, TC], f32, tag="pp", name=f"rbp{q}_{h}")
                nc.tensor.matmul(rbp[:], onesf_sb[:], rr[h][:],
                                 start=True, stop=True)
                rbc = nrmpool.tile([DH, TC], f32, tag="rbc",
                                   name=f"rbc{q}_{h}")
                nc.vector.tensor_copy(rbc[:], rbp[:])
                aTs[h] = atpool.tile([DH, TC], bf16, tag=f"aT{h}",
                                     name=f"aTq{q}_{h}")
                nc.vector.tensor_mul(aTs[h][:], u64[h][:], rbc[:])

            def pb_ship(q):
                _, _, aTs = nstate[q]
                for h in range(HPC):
                    nc.sync.dma_start(
                        out=a2a_in[q].rearrange(
                            "(j h p) t -> h p j t", j=N_CORES, h=HPC)[h],
                        in_=aTs[h][:].rearrange("p (j t) -> p j t", j=N_CORES))
                nc.gpsimd.collective_compute(
                    "AllToAll", mybir.AluOpType.bypass,
                    replica_groups=[list(range(N_CORES))],
                    ins=[a2a_in[q][:].opt()],
                    outs=[a2a_out[q][:].opt()],
                )
                del nstate[q]

            # output projection for chunk q, split into 4 chore pieces
            ostate = {}

            def op1(q):
                ap = oprpool.tile([128, ND * ROWS], bf16, tag="aprj",
                                  name=f"aprj{q}")
                nc.sync.dma_start(
                    out=ap[:].rearrange("p (d t) -> p d t", d=ND),
                    in_=a2a_out[q].rearrange("(d p) t -> p d t", p=128))
                oev = oprpool.tile([ROWS, D], f32, tag="oev", name=f"oev{q}")
                ostate[q] = (ap, oev, [None, None])

            def _op_mms(q, n):
                ap, oev, po = ostate[q]
                po[n] = pproj.tile([ROWS, 512], f32, tag="pp",
                                   name=f"po{q}_{n}")
                nsl = slice(512 * n, 512 * (n + 1))
                for d in range(ND):
                    nc.tensor.matmul(
                        po[n][:], ap[:, ROWS * d:ROWS * (d + 1)],
                        wo_sb[:, D * d + 512 * n:D * d + 512 * (n + 1)],
                        start=(d == 0),
                        stop=(not use_bias and d == ND - 1))
                if use_bias:
                    nc.tensor.matmul(po[n][:], ones_sb[:, 0:ROWS],
                                     bo_sb[:, nsl], start=False, stop=True)

            def op2(q):
                _op_mms(q, 0)

            def op3(q):
                ap, oev, po = ostate[q]
                nc.vector.tensor_copy(oev[:, 0:512], po[0][:])
                _op_mms(q, 1)

            def op4(q):
                ap, oev, po = ostate[q]
                nc.vector.tensor_copy(oev[:, 512:1024], po[1][:])
                nc.sync.dma_start(out=outs[q], in_=oev[:])
                del ostate[q]

            # ---- the unified s-tile stream -----------------------------
            # chores_pre[i] run before tile i's logits.  Tile i = chunk
            # i//16, s-tile i%16.  phase_a(q) is issued inline by pop_u
            # right after chunk q's last u-matmul so its PSUM buffers are
            # staged out before chunk q+1's first u-matmul reuses them.
            chores_pre = {
                2: [lambda: late_dma("k2")],
                4: [lambda: proj_chunk("k", 1)],
                6: [lambda: late_dma("k3")],
                8: [lambda: late_dma("rope1"), lambda: proj_chunk("k", 2)],
                12: [lambda: late_dma("rope2"), lambda: proj_chunk("k", 3)],
                14: [lambda: late_dma("rope3"), lambda: proj_chunk("q", 1)],
                16: [lambda: late_dma("wo")],
                21: [lambda: proj_chunk("q", 2)],
                25: [lambda: pb_h(0, 0)],
                26: [lambda: pb_h(0, 1)],
                27: [lambda: pb_ship(0)],
                34: [lambda: pb_h(1, 0)],
                35: [lambda: pb_h(1, 1)],
                36: [lambda: pb_ship(1)],
                37: [lambda: proj_chunk("q", 3)],
                40: [lambda: op1(0)],
                41: [lambda: op2(0)],
                42: [lambda: op3(0)],
                43: [lambda: op4(0)],
                46: [lambda: op1(1)],
                47: [lambda: op2(1)],
                48: [lambda: op3(1)],
                49: [lambda: op4(1)],
                50: [lambda: pb_h(2, 0)],
                51: [lambda: pb_h(2, 1)],
                52: [lambda: pb_ship(2)],
                61: [lambda: op1(2)],
                62: [lambda: op2(2)],
                63: [lambda: op3(2)],
            }

            proj_chunk("q", 0)
            proj_chunk("k", 0)

            pending = []  # (lag, chunk, s-tile, ex)

            def pop_u():
                _, qq, sl, exl = pending.pop(0)
                if qq == 0:
                    vproj(sl)
                u_mms(qq, sl, exl)
                if sl == NS - 1:
                    phase_a(qq)

            for i in range(NTC * NS):
                tc_i, s = divmod(i, NS)
                for fn in chores_pre.get(i, []):
                    fn()
                kt_t = kts[s // 4]
                ss = slice(128 * (s % 4), 128 * (s % 4 + 1))
                sp = pS.tile([128, 2 * TC], f32, tag="ps")
                nc.tensor.matmul(sp[:, 0:TC], kt_t[0:DH, ss],
                                 qts[tc_i][0:DH, :], start=True, stop=True)
                nc.tensor.matmul(sp[:, TC:2 * TC], kt_t[DH:128, ss],
                                 qts[tc_i][DH:128, :], start=True,
                                 stop=True, tile_position=(DH, 0))
                ex = expool.tile([128, 2 * TC], bf16, tag="ex")
                nc.scalar.activation(ex[:], sp[:], EXP, scale=0.125)
                # chunk 0 lags 8 tiles (v DMA still streaming); chunk q+1's
                # first tile lags 3 so phase_a(q)'s staging copies are done
                # before its u-matmul recycles the U psum buffers.
                lag = 8 if tc_i == 0 else (3 if s == 0 else 2)
                pending.append((lag, tc_i, s, ex))
                for _ in range(2):  # drain at most 2 deferred tiles
                    if pending and len(pending) > pending[0][0]:
                        pop_u()
                    else:
                        break
            while pending:
                pop_u()
            # drain: chunk 3 normalization + A2A + out-proj, chunk 2 tail
            pb_h(3, 0), pb_h(3, 1), pb_ship(3)
            op4(2)
            op1(3), op2(3), op3(3), op4(3)

    nc.compile()
    return nc


def _host_inputs(query, key, value, Wq, bq, Wk, bk, Wv, bv, Wo, bo):
    """Shard + lay out the full inputs for the 8 cores."""
    b = ml_dtypes.bfloat16
    qT = np.ascontiguousarray(query.T).astype(b)
    kT = np.ascontiguousarray(key.T).astype(b)
    vT = np.ascontiguousarray(value.T).astype(b)
    wo = Wo.astype(b)

    theta = 1.0 / (ROPE_BASE ** (np.arange(0, D, 2, dtype=np.float32) / D))
    idx = np.outer(np.arange(T, dtype=np.float32), theta)
    c, s = np.cos(idx), np.sin(idx)
    C = np.concatenate([c + s, c - s], axis=1).astype(np.float32)  # [T, D]

    in_maps = []
    for cidx in range(N_CORES):
        fs = slice(FPC * cidx, FPC * (cidx + 1))
        in_maps.append({
            "qT": qT, "kT": kT, "vT": vT,
            "wq": Wq[:, fs].astype(b), "wk": Wk[:, fs].astype(b),
            "wv": Wv[:, fs].astype(b), "wo": wo,
            "bq": bq[None, fs].astype(b), "bk": bk[None, fs].astype(b),
            "bv": bv[None, fs].astype(b), "bo": bo[None, :].astype(b),
            "ropeC": np.ascontiguousarray(C[:, fs].T),
        })
    return in_maps


def kernel(query, key, value, Wq, bq, Wk, bk, Wv, bv, Wo, bo, _trace=False):
    query, key, value = (np.asarray(x, np.float32) for x in (query, key, value))
    Wq, Wk, Wv, Wo = (np.asarray(x, np.float32) for x in (Wq, Wk, Wv, Wo))
    bq, bk, bv, bo = (np.asarray(x, np.float32) for x in (bq, bk, bv, bo))
    use_bias = any(np.any(b) for b in (bq, bk, bv, bo))
    ck = f"nc{int(use_bias)}"
    if ck not in _cache:
        _cache[ck] = _build(use_bias)
    nc = _cache[ck]
    in_maps = _host_inputs(query, key, value, Wq, bq, Wk, bk, Wv, bv, Wo, bo)
    res = run_bass_kernel_spmd(nc, in_maps, core_ids=list(range(N_CORES)),
                               trace=_trace)
    _cache["last_result"] = res
    out = np.empty((T, D), np.float32)
    for c in range(N_CORES):
        for q in range(NTC):
            r0 = TC * q + ROWS * c
            out[r0:r0 + ROWS, :] = res.results[c][f"out{q}"]
    return out


# revision 18
# speedup vs baseline: 1.1838x; 1.1838x over previous
"""Trainium2 Bass kernel for nn_MultiHeadAttention (dense transformer MHA).

Strategy (8-way tensor parallel over heads), v2:
  - Each of the 8 cores owns 2 heads (128 of the 1024 q/k/v features).
  - Host pre-transposes activations (query/key/value -> [D, T]), casts bf16;
    weights head-sliced per core (Wo full). RoPE is elementwise here
    (neg_half = [y1, -y2]) so it is one multiply by a host factor C^T.
  - Attention in transposed layout S^T[s, t]; unsafe softmax (exp on ACT,
    denominator via ones-column appended to V in the U matmul, normalize by
    partition-broadcast + multiply).
  - v2 scheduling: the TRN2 PE only reaches 2.4 GHz after ~3us of gapless
    execution and falls back to 1.2 GHz after any bubble, so the whole
    kernel is emitted as ONE continuous s-tile stream across all 4 t-chunks
    (64 tiles), with a lag-queue deferring each tile's U-matmuls a few tiles
    behind its logits (the ACT exp latency is hidden) and all other work
    (projections, chunk normalization, A2A ships, output projections)
    dropped into the stream as per-tile chores.  Inputs are DMA'd
    chunk-major (q/k) and s-major (v) on the two HWDGE rings so the first
    logits matmul can issue at ~9us instead of ~45us, and the PE is warmed
    up with throwaway matmuls until the first data lands.
  - Re-partition head-shard -> seq-shard with one AllToAll per 512-wide
    t-chunk; each core projects its own 64 rows per chunk through full Wo.
"""
import numpy as np
import ml_dtypes

import concourse.bass as bass
import concourse.mybir as mybir
import concourse.tile as tile
from concourse import bacc
from concourse.bass_utils import run_bass_kernel_spmd

# problem constants (hardcoded per contract)
T = 2048
D = 1024
H = 16
DH = 64
ROPE_BASE = 10000

N_CORES = 8
HPC = H // N_CORES          # heads per core = 2
FPC = HPC * DH              # features per core = 128
TC = 512                    # attention t-chunk
NTC = T // TC               # 4
NS = T // 128               # 16 s-tiles
ND = D // 128               # 8 d-tiles
VW = 2 * DH + 2             # 130: v_ext block width per s-tile
ROWS = TC // N_CORES        # 64 output rows per core per A2A chunk

bf16 = mybir.dt.bfloat16
f32 = mybir.dt.float32
EXP = mybir.ActivationFunctionType.Exp

_cache = {}


def _build(use_bias=True):
    nc = bacc.Bacc("TRN2", target_bir_lowering=False, debug=False,
                   num_devices=N_CORES)

    # ---- I/O -----------------------------------------------------------
    qT = nc.dram_tensor("qT", [D, T], bf16, kind="ExternalInput").ap()
    kT = nc.dram_tensor("kT", [D, T], bf16, kind="ExternalInput").ap()
    vT = nc.dram_tensor("vT", [D, T], bf16, kind="ExternalInput").ap()
    wq = nc.dram_tensor("wq", [D, FPC], bf16, kind="ExternalInput").ap()
    wk = nc.dram_tensor("wk", [D, FPC], bf16, kind="ExternalInput").ap()
    wv = nc.dram_tensor("wv", [D, FPC], bf16, kind="ExternalInput").ap()
    wo = nc.dram_tensor("wo", [D, D], bf16, kind="ExternalInput").ap()
    bq = nc.dram_tensor("bq", [1, FPC], bf16, kind="ExternalInput").ap()
    bk = nc.dram_tensor("bk", [1, FPC], bf16, kind="ExternalInput").ap()
    bv = nc.dram_tensor("bv", [1, FPC], bf16, kind="ExternalInput").ap()
    bo = nc.dram_tensor("bo", [1, D], bf16, kind="ExternalInput").ap()
    ropeC = nc.dram_tensor("ropeC", [FPC, T], f32, kind="ExternalInput").ap()
    outs = [nc.dram_tensor(f"out{q}", [ROWS, D], f32,
                           kind="ExternalOutput").ap() for q in range(NTC)]

    with tile.TileContext(nc) as tc:
        with (
            tc.tile_pool(name="win", bufs=1) as win,        # weights/consts
            tc.tile_pool(name="xin", bufs=1) as xin,        # input stream
            tc.tile_pool(name="qk", bufs=NTC) as qkpool,    # q^T / k^T
            tc.tile_pool(name="vx", bufs=NS) as vxpool,     # v_ext
            tc.tile_pool(name="ex", bufs=12) as expool,     # exp(S^T)
            tc.tile_pool(name="at", bufs=1) as atpool,      # attn^T halves
            tc.tile_pool(name="nrm", bufs=4) as nrmpool,    # u_sb / Rbc
            tc.tile_pool(name="opr", bufs=2) as oprpool,    # out-proj tiles
            tc.tile_pool(name="pp", bufs=2, space="PSUM") as pproj,
            tc.tile_pool(name="ps", bufs=2, space="PSUM") as pS,
            tc.tile_pool(name="pu", bufs=2, space="PSUM") as pU,
            tc.tile_pool(name="dram", bufs=1, space="DRAM") as dram,
        ):
            # ---- constants / weights / inputs, in consumption order ----
            wq_sb = win.tile([128, ND * FPC], bf16, tag="wq")
            wk_sb = win.tile([128, ND * FPC], bf16, tag="wk")
            wv_sb = win.tile([128, ND * FPC], bf16, tag="wv")
            bq_sb = win.tile([1, FPC], bf16, tag="bq")
            bk_sb = win.tile([1, FPC], bf16, tag="bk")
            bv_sb = win.tile([1, FPC], bf16, tag="bv")
            bo_sb = win.tile([1, D], bf16, tag="bo")
            ropes = [win.tile([FPC, TC], f32, tag="rope", bufs=NTC,
                              name=f"rope{i}") for i in range(NTC)]
            ones_sb = win.tile([1, T], bf16, tag="ones")
            nc.gpsimd.memset(ones_sb[:], 1.0)
            onesf_sb = win.tile([1, DH], f32, tag="onesf")
            nc.gpsimd.memset(onesf_sb[:], 1.0)
            # preload the EXP activation table so the first real exp in the
            # s-stream doesn't eat the ~1.3us table load.
            pre_sb = win.tile([1, 2], f32, tag="pre")
            nc.scalar.activation(pre_sb[:], onesf_sb[:, 0:2], EXP)
            qin = xin.tile([128, ND * T], bf16, tag="qin")
            kin = xin.tile([128, ND * T], bf16, tag="kin")
            vin = xin.tile([128, ND * T], bf16, tag="vin")

            # ---- input DMA, chunk-major so compute starts early --------
            # ring A = SP (sync), ring B = ACT (scalar); they drain
            # concurrently.  k gets a dedicated ring: every s-tile of chunk
            # c's logits needs k-chunk s//4, q/v/weights share ring A.
            def _wdma(eng, w_sb, w):
                eng.dma_start(
                    out=w_sb[:].rearrange("p (d m) -> p d m", d=ND),
                    in_=w.rearrange("(d p) m -> p d m", p=128))

            def _xchunk(eng, x_sb, x, c, nch=1):
                # one 512-wide column chunk (nch of them) of all 8 d-tiles,
                # as a SINGLE dma_start: keeping the per-ring outstanding-DMA
                # count low avoids issue-stalls on the initiating engine.
                cs = slice(TC * c, TC * (c + nch))
                eng.dma_start(
                    out=x_sb[:].rearrange("p (d m) -> p d m", d=ND)[:, :, cs],
                    in_=x.rearrange("(d p) m -> p d m", p=128)[:, :, cs])

            # ring B (ACT) carries ONLY 4 DMAs (wk, bk, k0, k1): the tile
            # framework flow-controls in-flight DMAs with semaphore chains,
            # so a longer ACT prologue would block the exp activations
            # queued behind it on the ACT engine.  Ring A (SP) carries
            # everything else, ordered by first need; issue-stalls on the
            # sync engine are harmless because nothing time-critical
            # (A2A ships run mid-stream, out-proj loads in the drain)
            # queues there until the ring has drained.
            _wdma(nc.scalar, wk_sb, wk)
            nc.scalar.dma_start(out=bk_sb[:], in_=bk)
            _xchunk(nc.scalar, kin, kT, 0)
            _xchunk(nc.scalar, kin, kT, 1)
            wo_sb = win.tile([128, ND * D], bf16, tag="wo")
            _wdma(nc.sync, wq_sb, wq)
            nc.sync.dma_start(out=bq_sb[:], in_=bq)
            nc.sync.dma_start(out=ropes[0][:], in_=ropeC[:, 0:TC])
            _xchunk(nc.sync, qin, qT, 0)
            _wdma(nc.sync, wv_sb, wv)
            nc.sync.dma_start(out=bv_sb[:], in_=bv)
            _xchunk(nc.sync, vin, vT, 0)
            _xchunk(nc.sync, kin, kT, 2)
            _xchunk(nc.sync, kin, kT, 3)
            _xchunk(nc.sync, vin, vT, 1)
            _xchunk(nc.sync, qin, qT, 1)
            nc.sync.dma_start(out=ropes[1][:], in_=ropeC[:, TC:2 * TC])
            _xchunk(nc.sync, vin, vT, 2)
            nc.sync.dma_start(out=ropes[2][:], in_=ropeC[:, 2 * TC:3 * TC])
            _xchunk(nc.sync, vin, vT, 3)
            _xchunk(nc.sync, qin, qT, 2)
            nc.sync.dma_start(out=ropes[3][:], in_=ropeC[:, 3 * TC:4 * TC])
            _xchunk(nc.sync, qin, qT, 3)
            nc.sync.dma_start(
                out=wo_sb[:].rearrange("p (d m) -> p d m", d=ND),
                in_=wo.rearrange("(d p) m -> p d m", p=128))
            nc.sync.dma_start(out=bo_sb[:], in_=bo)

            # PE warmup: back-to-back matmuls until the first inputs land;
            # keeps the DVFS ramp running so projections start at speed.
            wup = pproj.tile([DH, 512], f32, tag="pp", name="wup")
            for _ in range(10):
                nc.tensor.matmul(wup[:], ones_sb[:, 0:DH], ones_sb[:, 0:512],
                                 start=True, stop=True)
            # consume the warmup result (it is exactly 1.0) so DCE keeps it
            nc.vector.tensor_copy(ones_sb[:, 0:512], wup[0:1, :])

            # ---- projections (per 512-wide chunk, chore-schedulable) ---
            qts = [qkpool.tile([128, TC], bf16, tag="qt", name=f"qt{i}")
                   for i in range(NTC)]
            kts = [qkpool.tile([128, TC], bf16, tag="kt", name=f"kt{i}")
                   for i in range(NTC)]

            def proj_chunk(which, c):
                x_sb, w_sb, b_sb, x_in = {
                    "q": (qts[c], wq_sb, bq_sb, qin),
                    "k": (kts[c], wk_sb, bk_sb, kin),
                }[which]
                ts = slice(TC * c, TC * (c + 1))
                ps = pproj.tile([128, TC], f32, tag="pp",
                                name=f"pj_{which}{c}")
                for d in range(ND):
                    nc.tensor.matmul(
                        ps[:], w_sb[:, FPC * d:FPC * (d + 1)],
                        x_in[:, T * d + TC * c:T * d + TC * (c + 1)],
                        start=(d == 0),
                        stop=(not use_bias and d == ND - 1))
                if use_bias:
                    nc.tensor.matmul(ps[:], b_sb[:], ones_sb[:, ts],
                                     start=False, stop=True)
                nc.vector.tensor_mul(x_sb[:], ps[:], ropes[c][:])

            # v_ext: 16 tiles [128, VW]; block: [v_h0 | ones | v_h1 | ones]
            vs = [vxpool.tile([128, VW], bf16, tag="vext", name=f"vext{s}")
                  for s in range(NS)]
            for s in range(NS):
                nc.gpsimd.memset(vs[s][:, DH::DH + 1], 1.0)  # ones columns

            def vproj(s):
                ps = pproj.tile([128, FPC], f32, tag="pp", name=f"vps{s}")
                for d in range(ND):
                    nc.tensor.matmul(
                        ps[:], vin[:, T * d + 128 * s:T * d + 128 * (s + 1)],
                        wv_sb[:, FPC * d:FPC * (d + 1)],
                        start=(d == 0),
                        stop=(not use_bias and d == ND - 1))
                if use_bias:
                    nc.tensor.matmul(ps[:], ones_sb[:, 0:128], bv_sb[:],
                                     start=False, stop=True)
                nc.vector.tensor_copy(
                    vs[s][:].rearrange("p (h w) -> p h w", h=2)[:, :, 0:DH],
                    ps.rearrange("p (h w) -> p h w", h=2))

            # ---- A2A bounce buffers ------------------------------------
            a2a_in = [dram.tile([8 * 128, ROWS], bf16, tag=f"a2ai{i}",
                                name=f"a2a_in{i}") for i in range(NTC)]
            a2a_out = [dram.tile([8 * 128, ROWS], bf16, tag=f"a2ao{i}",
                                 name=f"a2a_out{i}") for i in range(NTC)]

            # per-chunk U psum tiles, created lazily at first u-matmul
            ups = {}

            def u_mms(q, sl, ex):
                if q not in ups:
                    ups[q] = [pU.tile([DH + 1, TC], f32, tag="pu",
                                      name=f"up{q}_{h}") for h in range(HPC)]
                for h in range(HPC):
                    o = (DH + 1) * h
                    nc.tensor.matmul(
                        ups[q][h][:], vs[sl][:, o:o + DH + 1],
                        ex[:, TC * h:TC * (h + 1)],
                        start=(sl == 0), stop=(sl == NS - 1))

            # phase A: stage U and 1/colsum to SBUF, freeing PSUM slots
            nstate = {}

            def phase_a(q):
                u64, rr = [], []
                for h in range(HPC):
                    u_sb = nrmpool.tile([DH, TC], f32, tag="u64",
                                        name=f"u64_{q}_{h}")
                    nc.vector.tensor_copy(u_sb[:], ups[q][h][0:DH, :])
                    r_sb = nrmpool.tile([1, TC], f32, tag="rsb",
                                        name=f"rsb{q}_{h}")
                    nc.vector.tensor_copy(r_sb[:], ups[q][h][DH:DH + 1, :])
                    nc.vector.reciprocal_approx_fast(r_sb[:], r_sb[:])
                    u64.append(u_sb)
                    rr.append(r_sb)
                del ups[q]
                nstate[q] = (u64, rr, [None, None])

            # phase B: normalize per head (partition-broadcast of 1/sum via
            # a rank-1 matmul), then ship to the bounce + trigger the A2A.
            def pb_h(q, h):
                u64, rr, aTs = nstate[q]
                rbp = pproj.tile([DH, TC], f32, tag="pp", name=f"rbp{q}_{h}")
                nc.tensor.matmul(rbp[:], onesf_sb[:], rr[h][:],
                                 start=True, stop=True)
                rbc = nrmpool.tile([DH, TC], f32, tag="rbc",
                                   name=f"rbc{q}_{h}")
                nc.vector.tensor_copy(rbc[:], rbp[:])
                aTs[h] = atpool.tile([DH, TC], bf16, tag=f"aT{h}",
                                     name=f"aTq{q}_{h}")
                nc.vector.tensor_mul(aTs[h][:], u64[h][:], rbc[:])

            def pb_ship(q):
                _, _, aTs = nstate[q]
                for h in range(HPC):
                    nc.sync.dma_start(
                        out=a2a_in[q].rearrange(
                            "(j h p) t -> h p j t", j=N_CORES, h=HPC)[h],
                        in_=aTs[h][:].rearrange("p (j t) -> p j t", j=N_CORES))
                nc.gpsimd.collective_compute(
                    "AllToAll", mybir.AluOpType.bypass,
                    replica_groups=[list(range(N_CORES))],
                    ins=[a2a_in[q][:].opt()],
                    outs=[a2a_out[q][:].opt()],
                )
                del nstate[q]

            # output projection for chunk q, split into 4 chore pieces
            ostate = {}

            def op1(q):
                ap = oprpool.tile([128, ND * ROWS], bf16, tag="aprj",
                                  name=f"aprj{q}")
                nc.sync.dma_start(
                    out=ap[:].rearrange("p (d t) -> p d t", d=ND),
                    in_=a2a_out[q].rearrange("(d p) t -> p d t", p=128))
                oev = oprpool.tile([ROWS, D], f32, tag="oev", name=f"oev{q}")
                ostate[q] = (ap, oev, [None, None])

            def _op_mms(q, n):
                ap, oev, po = ostate[q]
                po[n] = pproj.tile([ROWS, 512], f32, tag="pp",
                                   name=f"po{q}_{n}")
                nsl = slice(512 * n, 512 * (n + 1))
                for d in range(ND):
                    nc.tensor.matmul(
                        po[n][:], ap[:, ROWS * d:ROWS * (d + 1)],
                        wo_sb[:, D * d + 512 * n:D * d + 512 * (n + 1)],
                        start=(d == 0),
                        stop=(not use_bias and d == ND - 1))
                if use_bias:
                    nc.tensor.matmul(po[n][:], ones_sb[:, 0:ROWS],
                                     bo_sb[:, nsl], start=False, stop=True)

            def op2(q):
                _op_mms(q, 0)

            def op3(q):
                ap, oev, po = ostate[q]
                nc.vector.tensor_copy(oev[:, 0:512], po[0][:])
                _op_mms(q, 1)

            def op4(q):
                ap, oev, po = ostate[q]
                nc.vector.tensor_copy(oev[:, 512:1024], po[1][:])
                nc.sync.dma_start(out=outs[q], in_=oev[:])
                del ostate[q]

            # ---- the unified s-tile stream -----------------------------
            # chores_pre[i] run before tile i's logits.  Tile i = chunk
            # i//16, s-tile i%16.  phase_a(q) is issued inline by pop_u
            # right after chunk q's last u-matmul so its PSUM buffers are
            # staged out before chunk q+1's first u-matmul reuses them.
            chores_pre = {
                4: [lambda: proj_chunk("k", 1)],
                8: [lambda: proj_chunk("k", 2)],
                12: [lambda: proj_chunk("k", 3)],
                14: [lambda: proj_chunk("q", 1)],
                22: [lambda: proj_chunk("q", 2)],
                26: [lambda: pb_h(0, 0)],
                27: [lambda: pb_h(0, 1)],
                28: [lambda: pb_ship(0)],
                34: [lambda: pb_h(1, 0)],
                35: [lambda: pb_h(1, 1)],
                36: [lambda: pb_ship(1)],
                37: [lambda: proj_chunk("q", 3)],
                50: [lambda: pb_h(2, 0)],
                51: [lambda: pb_h(2, 1)],
                52: [lambda: pb_ship(2)],
            }

            proj_chunk("q", 0)
            proj_chunk("k", 0)

            pending = []  # (lag, chunk, s-tile, ex)

            def pop_u():
                _, qq, sl, exl = pending.pop(0)
                if qq == 0:
                    vproj(sl)
                u_mms(qq, sl, exl)
                if sl == NS - 1:
                    phase_a(qq)

            for i in range(NTC * NS):
                tc_i, s = divmod(i, NS)
                for fn in chores_pre.get(i, []):
                    fn()
                kt_t = kts[s // 4]
                ss = slice(128 * (s % 4), 128 * (s % 4 + 1))
                sp = pS.tile([128, 2 * TC], f32, tag="ps")
                nc.tensor.matmul(sp[:, 0:TC], kt_t[0:DH, ss],
                                 qts[tc_i][0:DH, :], start=True, stop=True)
                nc.tensor.matmul(sp[:, TC:2 * TC], kt_t[DH:128, ss],
                                 qts[tc_i][DH:128, :], start=True,
                                 stop=True, tile_position=(DH, 0))
                ex = expool.tile([128, 2 * TC], bf16, tag="ex")
                nc.scalar.activation(ex[:], sp[:], EXP, scale=0.125)
                # chunk 0 lags 10 tiles (v DMA still streaming); chunk q+1's
                # first tile lags 3 so phase_a(q)'s staging copies are done
                # before its u-matmul recycles the U psum buffers.
                lag = 10 if tc_i == 0 else (3 if s == 0 else 2)
                pending.append((lag, tc_i, s, ex))
                for _ in range(2):  # drain at most 2 deferred tiles
                    if pending and len(pending) > pending[0][0]:
                        pop_u()
                    else:
                        break
            while pending:
                pop_u()
            # drain: chunk 3 normalization + its A2A, then ALL four output
            # projections back-to-back — chunks 0-2's A2As landed long ago
            # and their PE work hides A2A(3)'s flight time, so no mid-
            # stream deadline ever couples the s-stream to a collective.
            pb_h(3, 0), pb_h(3, 1), pb_ship(3)
            op1(0), op1(1)
            for q in range(NTC):
                op2(q), op3(q)
                if q + 2 < NTC:
                    op1(q + 2)
                op4(q)

    nc.compile()
    return nc


def _host_inputs(query, key, value, Wq, bq, Wk, bk, Wv, bv, Wo, bo):
    """Shard + lay out the full inputs for the 8 cores."""
    b = ml_dtypes.bfloat16
    qT = np.ascontiguousarray(query.T).astype(b)
    kT = np.ascontiguousarray(key.T).astype(b)
    vT = np.ascontiguousarray(value.T).astype(b)
    wo = Wo.astype(b)

    theta = 1.0 / (ROPE_BASE ** (np.arange(0, D, 2, dtype=np.float32) / D))
    idx = np.outer(np.arange(T, dtype=np.float32), theta)
    c, s = np.cos(idx), np.sin(idx)
    C = np.concatenate([c + s, c - s], axis=1).astype(np.float32)  # [T, D]

    in_maps = []
    for cidx in range(N_CORES):
        fs = slice(FPC * cidx, FPC * (cidx + 1))
        in_maps.append({
            "qT": qT, "kT": kT, "vT": vT,
            "wq": Wq[:, fs].astype(b), "wk": Wk[:, fs].astype(b),
            "wv": Wv[:, fs].astype(b), "wo": wo,
            "bq": bq[None, fs].astype(b), "bk": bk[None, fs].astype(b),
            "bv": bv[None, fs].astype(b), "bo": bo[None, :].astype(b),
            "ropeC": np.ascontiguousarray(C[:, fs].T),
        })
    return in_maps


def kernel(query, key, value, Wq, bq, Wk, bk, Wv, bv, Wo, bo, _trace=False):
    query, key, value = (np.asarray(x, np.float32) for x in (query, key, value))
    Wq, Wk, Wv, Wo = (np.asarray(x, np.float32) for x in (Wq, Wk, Wv, Wo))
    bq, bk, bv, bo = (np.asarray(x, np.float32) for x in (bq, bk, bv, bo))
    use_bias = any(np.any(b) for b in (bq, bk, bv, bo))
    ck = f"nc{int(use_bias)}"
    if ck not in _cache:
        _cache[ck] = _build(use_bias)
    nc = _cache[ck]
    in_maps = _host_inputs(query, key, value, Wq, bq, Wk, bk, Wv, bv, Wo, bo)
    res = run_bass_kernel_spmd(nc, in_maps, core_ids=list(range(N_CORES)),
                               trace=_trace)
    _cache["last_result"] = res
    out = np.empty((T, D), np.float32)
    for c in range(N_CORES):
        for q in range(NTC):
            r0 = TC * q + ROWS * c
            out[r0:r0 + ROWS, :] = res.results[c][f"out{q}"]
    return out
